# revision 16
# baseline (speedup 1.0000x reference)
"""MoE layer (top-2 of 8 experts, H=768, F=3072, T=4096) on 8 TRN2 NeuronCores.

Strategy: expert parallelism with sparse dispatch.
  - Host computes the gate exactly as the reference does (jax on CPU:
    logits -> softmax -> top-2 -> renormalized top-2 weights) and builds the
    per-expert token lists ("all-to-all token dispatch" done host-side).
  - Core e receives: the tokens routed to expert e (gathered, transposed to
    [H, C] so both GEMMs need no on-device transposes), that expert's W1/W2,
    and the per-token combine weight.
  - Device computes y_e = (silu(x_e @ W1_e) @ W2_e) * w_tok. GEMM1 runs in
    fp32r (full PE rate, ~1e-4 rel err), GEMM2 in bf16. Since the combine
    weight is a per-token scalar it commutes past W2 and is applied to the
    GEMM2 output on the scalar engine.
  - Host scatter-adds the two expert partials per token (combine/unshard).
"""

import os
import sys
from contextlib import ExitStack

for _p in ("/opt/trn_rl_repo",):
    if _p not in sys.path and os.path.isdir(_p):
        sys.path.insert(0, _p)

import numpy as np

B, S, H, F, E, TOPK = 2, 2048, 768, 3072, 8, 2
T = B * S
KH = H // 128   # 6  contraction chunks for GEMM1
KF = F // 128   # 24 contraction chunks for GEMM2
CHUNK = 384     # token chunk (moving-dim) for GEMM1
G1_BF16 = True  # GEMM1 in bf16 (vs fp32r)

_COMPILED = {}  # C -> (nc,)


def _route(x_flat: np.ndarray, Wg: np.ndarray):
    """Gate computed with the same ops/platform as the reference (jax CPU)."""
    import jax
    import jax.numpy as jnp

    cpu = jax.devices("cpu")[0]
    with jax.default_device(cpu):
        logits = jnp.asarray(x_flat) @ jnp.asarray(Wg)
        probs = jax.nn.softmax(logits, axis=-1)
        top_scores, top_idx = jax.lax.top_k(probs, TOPK)
        top_w = jax.nn.softmax(top_scores.astype(jnp.float32), axis=-1)
        return (np.asarray(top_idx), np.asarray(top_w, dtype=np.float32))


def _build(C: int):
    import concourse.tile as tile
    from concourse import bacc, mybir

    F32 = mybir.dt.float32
    F32R = mybir.dt.float32r
    BF16 = mybir.dt.bfloat16
    AF = mybir.ActivationFunctionType

    G1 = BF16 if G1_BF16 else F32R
    G1S = BF16 if G1_BF16 else F32  # storage dtype in DRAM

    nc = bacc.Bacc("TRN2", target_bir_lowering=False, debug=False)
    xg = nc.dram_tensor("xg", [H, C], G1S, kind="ExternalInput").ap()
    wt = nc.dram_tensor("wt", [-(-C // 128) * 128, 1], F32, kind="ExternalInput").ap()
    w1 = nc.dram_tensor("w1", [H, F], G1S, kind="ExternalInput").ap()
    w2 = nc.dram_tensor("w2", [F, H], BF16, kind="ExternalInput").ap()
    y = nc.dram_tensor("y", [C, H], F32, kind="ExternalOutput").ap()

    def g1cast(ap):
        return ap if G1_BF16 else ap.bitcast(F32R)

    NTOK = -(-C // 128)  # 128-tile count (last tile may be partial)
    chunks = []
    c0 = 0
    while c0 < C:
        csz = min(CHUNK, C - c0)
        chunks.append((c0, csz))
        c0 += csz

    with tile.TileContext(nc) as tc, ExitStack() as ctx:
        w1p = ctx.enter_context(tc.tile_pool(name="w1p", bufs=1))
        w2p = ctx.enter_context(tc.tile_pool(name="w2p", bufs=1))
        xp = ctx.enter_context(tc.tile_pool(name="xp", bufs=3))
        hp = ctx.enter_context(tc.tile_pool(name="hp", bufs=3))
        yp = ctx.enter_context(tc.tile_pool(name="yp", bufs=3))
        wtp = ctx.enter_context(tc.tile_pool(name="wtp", bufs=1))
        ps1 = ctx.enter_context(tc.tile_pool(name="ps1", bufs=4, space="PSUM"))
        ps2 = ctx.enter_context(tc.tile_pool(name="ps2", bufs=2, space="PSUM"))

        # x chunk 0 first, then W1 in f-eighths (chunk-0 GEMM1 starts after
        # only x0 + the first eighth of W1 has landed), x1 after the first
        # two eighths.
        xts = [xp.tile([128, KH, csz], G1, tag="x", name=f"x_{ci}")
               for ci, (c0, csz) in enumerate(chunks)]

        def load_x(ci):
            c0, csz = chunks[ci]
            nc.sync.dma_start(
                xts[ci][:],
                g1cast(xg[:, c0:c0 + csz]
                       .rearrange("(ko p) n -> p ko n", p=128)),
            )

        load_x(0)
        w1t = [w1p.tile([128, F], G1, tag=f"w1_{k}", name=f"w1_{k}")
               for k in range(KH)]
        for q in range(8):
            fsl = slice(q * (F // 8), (q + 1) * (F // 8))
            for k in range(KH):
                nc.sync.dma_start(
                    w1t[k][:, fsl],
                    g1cast(w1[k * 128:(k + 1) * 128, fsl]),
                )
            if q == 1 and len(chunks) > 1:
                load_x(1)
        # all combine weights: wta[p, n] = wt[n*128 + p]
        wta = wtp.tile([128, NTOK], F32, tag="wta")
        nc.sync.dma_start(wta[:], wt.rearrange("(n p) one -> p (n one)", p=128))
        w2t = [w2p.tile([128, H], BF16, tag=f"w2_{k}", name=f"w2_{k}")
               for k in range(KF)]
        for hh in range(2):
            hsl = slice(hh * 384, (hh + 1) * 384)
            for k in range(KF):
                nc.sync.dma_start(w2t[k][:, hsl], w2[k * 128:(k + 1) * 128, hsl])

        for ci, (c0, csz) in enumerate(chunks):
            xt = xts[ci]
            if ci >= 2:
                load_x(ci)
            ht = hp.tile([128, KF, csz], BF16, tag="h")
            # GEMM1: hT[f, c] = silu(sum_k W1[k,f]^T xg[k,c])
            for f in range(KF):
                ps = ps1.tile([128, csz], mybir.dt.float32, tag="ps1")
                for k in range(KH):
                    nc.tensor.matmul(
                        ps[:],
                        w1t[k][:, f * 128:(f + 1) * 128],
                        xt[:, k, :],
                        start=(k == 0),
                        stop=(k == KH - 1),
                    )
                nc.scalar.activation(ht[:, f, :], ps[:], AF.Silu)
            # GEMM2: y[c, :] = (hT^T @ W2) * w_tok
            for m in range(-(-csz // 128)):
                mt = c0 // 128 + m
                mw = min(128, csz - m * 128)   # partial last token-tile
                msl = slice(m * 128, m * 128 + mw)
                pa = ps2.tile([128, 384], mybir.dt.float32, tag="psA")
                pb = ps2.tile([128, 384], mybir.dt.float32, tag="psB")
                for k in range(KF):
                    nc.tensor.matmul(pa[:mw, :], ht[:, k, msl],
                                     w2t[k][:, 0:384],
                                     start=(k == 0), stop=(k == KF - 1))
                for k in range(KF):
                    nc.tensor.matmul(pb[:mw, :], ht[:, k, msl],
                                     w2t[k][:, 384:768],
                                     start=(k == 0), stop=(k == KF - 1))
                yt = yp.tile([128, H], mybir.dt.float32, tag="y")
                nc.scalar.activation(yt[:mw, 0:384], pa[:mw, :], AF.Copy,
                                     scale=wta[:mw, mt:mt + 1])
                nc.scalar.activation(yt[:mw, 384:768], pb[:mw, :], AF.Copy,
                                     scale=wta[:mw, mt:mt + 1])
                nc.sync.dma_start(y[c0 + m * 128:c0 + m * 128 + mw, :],
                                  yt[:mw, :])

    nc.compile()
    return nc


def kernel(x: np.ndarray, Wg: np.ndarray, W1: np.ndarray, W2: np.ndarray):
    import ml_dtypes
    from concourse.bass_utils import run_bass_kernel_spmd

    x = np.asarray(x, dtype=np.float32)
    Wg = np.asarray(Wg, dtype=np.float32)
    W1 = np.asarray(W1, dtype=np.float32)
    W2 = np.asarray(W2, dtype=np.float32)
    x_flat = np.ascontiguousarray(x.reshape(T, H))

    top_idx, top_w = _route(x_flat, Wg)

    idx_lists = []
    wt_lists = []
    for e in range(E):
        sel = top_idx == e                       # [T, K] bool
        tok = np.nonzero(sel.any(axis=1))[0]     # tokens routed to e
        w_tok = (top_w * sel).sum(axis=1)[tok].astype(np.float32)
        idx_lists.append(tok)
        wt_lists.append(w_tok)

    max_cnt = max(len(t) for t in idx_lists)
    C = max(256, max_cnt)

    if C not in _COMPILED:
        _COMPILED[C] = _build(C)
    nc = _COMPILED[C]

    in_maps = []
    for e in range(E):
        tok = idx_lists[e]
        cnt = len(tok)
        g1dt = ml_dtypes.bfloat16 if G1_BF16 else np.float32
        xg = np.zeros((H, C), dtype=g1dt)
        xg[:, :cnt] = x_flat[tok].T.astype(g1dt)
        wt = np.zeros((-(-C // 128) * 128, 1), dtype=np.float32)
        wt[:cnt, 0] = wt_lists[e]
        in_maps.append({
            "xg": xg,
            "wt": wt,
            "w1": W1[e].astype(g1dt),
            "w2": W2[e].astype(ml_dtypes.bfloat16),
        })

    res = run_bass_kernel_spmd(
        nc, in_maps, core_ids=list(range(E)),
        trace=bool(globals().get("TRACE", False)),
    )
    globals()["LAST_RESULT"] = res

    y = np.zeros((T, H), dtype=np.float32)
    for e in range(E):
        tok = idx_lists[e]
        y[tok] += res.results[e]["y"][:len(tok)]

    return y.reshape(B, S, H), np.zeros((), dtype=np.float32)


# revision 18
# speedup vs baseline: 1.0164x; 1.0164x over previous
"""MoE layer (top-2 of 8 experts, H=768, F=3072, T=4096) on 8 TRN2 NeuronCores.

Strategy: expert parallelism with sparse dispatch.
  - Host computes the gate exactly as the reference does (jax on CPU:
    logits -> softmax -> top-2 -> renormalized top-2 weights) and builds the
    per-expert token lists ("all-to-all token dispatch" done host-side).
  - Core e receives: the tokens routed to expert e (gathered, transposed to
    [H, C] so both GEMMs need no on-device transposes), that expert's W1/W2,
    and the per-token combine weight.
  - Device computes y_e = (silu(x_e @ W1_e) @ W2_e) * w_tok. Both GEMMs run
    in bf16 (fp32 PSUM accumulation; G1_BF16=False switches GEMM1 to fp32r).
    Since the combine weight is a per-token scalar it commutes past W2 and is
    applied to the GEMM2 output on the scalar engine.
  - Host scatter-adds the two expert partials per token (combine/unshard).
"""

import os
import sys
from contextlib import ExitStack

for _p in ("/opt/trn_rl_repo",):
    if _p not in sys.path and os.path.isdir(_p):
        sys.path.insert(0, _p)

import numpy as np

B, S, H, F, E, TOPK = 2, 2048, 768, 3072, 8, 2
T = B * S
KH = H // 128   # 6  contraction chunks for GEMM1
KF = F // 128   # 24 contraction chunks for GEMM2
CHUNK = 384     # token chunk (moving-dim) for GEMM1
G1_BF16 = True  # GEMM1 in bf16 (vs fp32r)

_COMPILED = {}  # C -> (nc,)


def _route(x_flat: np.ndarray, Wg: np.ndarray):
    """Gate computed with the same ops/platform as the reference (jax CPU)."""
    try:
        import jax
        import jax.numpy as jnp

        cpu = jax.devices("cpu")[0]
        with jax.default_device(cpu):
            logits = jnp.asarray(x_flat) @ jnp.asarray(Wg)
            probs = jax.nn.softmax(logits, axis=-1)
            top_scores, top_idx = jax.lax.top_k(probs, TOPK)
            top_w = jax.nn.softmax(top_scores.astype(jnp.float32), axis=-1)
            return (np.asarray(top_idx), np.asarray(top_w, dtype=np.float32))
    except Exception:
        # numpy fallback (identical math; only fp summation order differs)
        logits = x_flat @ Wg
        z = logits - logits.max(axis=-1, keepdims=True)
        p = np.exp(z)
        probs = p / p.sum(axis=-1, keepdims=True)
        # top-k with ties broken toward lower index, like jax.lax.top_k
        order = np.argsort(-probs, axis=-1, kind="stable")
        top_idx = order[:, :TOPK].astype(np.int32)
        top_scores = np.take_along_axis(probs, top_idx, axis=-1)
        z2 = top_scores - top_scores.max(axis=-1, keepdims=True)
        p2 = np.exp(z2)
        top_w = (p2 / p2.sum(axis=-1, keepdims=True)).astype(np.float32)
        return top_idx, top_w


def _build(C: int):
    import concourse.tile as tile
    from concourse import bacc, mybir

    F32 = mybir.dt.float32
    F32R = mybir.dt.float32r
    BF16 = mybir.dt.bfloat16
    AF = mybir.ActivationFunctionType

    G1 = BF16 if G1_BF16 else F32R
    G1S = BF16 if G1_BF16 else F32  # storage dtype in DRAM

    nc = bacc.Bacc("TRN2", target_bir_lowering=False, debug=False)
    xg = nc.dram_tensor("xg", [H, C], G1S, kind="ExternalInput").ap()
    wt = nc.dram_tensor("wt", [-(-C // 128) * 128, 1], F32, kind="ExternalInput").ap()
    w1 = nc.dram_tensor("w1", [H, F], G1S, kind="ExternalInput").ap()
    w2 = nc.dram_tensor("w2", [F, H], BF16, kind="ExternalInput").ap()
    y = nc.dram_tensor("y", [C, H], F32, kind="ExternalOutput").ap()

    def g1cast(ap):
        return ap if G1_BF16 else ap.bitcast(F32R)

    NTOK = -(-C // 128)  # 128-tile count (last tile may be partial)
    chunks = []
    c0 = 0
    while c0 < C:
        csz = min(CHUNK, C - c0)
        chunks.append((c0, csz))
        c0 += csz

    with tile.TileContext(nc) as tc, ExitStack() as ctx:
        w1p = ctx.enter_context(tc.tile_pool(name="w1p", bufs=1))
        w2p = ctx.enter_context(tc.tile_pool(name="w2p", bufs=1))
        xp = ctx.enter_context(tc.tile_pool(name="xp", bufs=3))
        hp = ctx.enter_context(tc.tile_pool(name="hp", bufs=3))
        yp = ctx.enter_context(tc.tile_pool(name="yp", bufs=3))
        wtp = ctx.enter_context(tc.tile_pool(name="wtp", bufs=1))
        ps1 = ctx.enter_context(tc.tile_pool(name="ps1", bufs=4, space="PSUM"))
        ps2 = ctx.enter_context(tc.tile_pool(name="ps2", bufs=2, space="PSUM"))

        # x chunk 0 first, then W1 in f-eighths (chunk-0 GEMM1 starts after
        # only x0 + the first eighth of W1 has landed), x1 after the first
        # two eighths.
        xts = [xp.tile([128, KH, csz], G1, tag="x", name=f"x_{ci}")
               for ci, (c0, csz) in enumerate(chunks)]

        def load_x(ci):
            c0, csz = chunks[ci]
            nc.sync.dma_start(
                xts[ci][:],
                g1cast(xg[:, c0:c0 + csz]
                       .rearrange("(ko p) n -> p ko n", p=128)),
            )

        load_x(0)
        w1t = [w1p.tile([128, F], G1, tag=f"w1_{k}", name=f"w1_{k}")
               for k in range(KH)]
        for q in range(8):
            fsl = slice(q * (F // 8), (q + 1) * (F // 8))
            for k in range(KH):
                nc.sync.dma_start(
                    w1t[k][:, fsl],
                    g1cast(w1[k * 128:(k + 1) * 128, fsl]),
                )
            if q == 1 and len(chunks) > 1:
                load_x(1)
        # all combine weights: wta[p, n] = wt[n*128 + p]
        wta = wtp.tile([128, NTOK], F32, tag="wta")
        nc.sync.dma_start(wta[:], wt.rearrange("(n p) one -> p (n one)", p=128))
        w2t = [w2p.tile([128, H], BF16, tag=f"w2_{k}", name=f"w2_{k}")
               for k in range(KF)]
        for hh in range(2):
            hsl = slice(hh * 384, (hh + 1) * 384)
            for k in range(KF):
                nc.sync.dma_start(w2t[k][:, hsl], w2[k * 128:(k + 1) * 128, hsl])

        for ci, (c0, csz) in enumerate(chunks):
            xt = xts[ci]
            if ci >= 2:
                load_x(ci)
            ht = hp.tile([128, KF, csz], BF16, tag="h")
            # GEMM1: hT[f, c] = silu(sum_k W1[k,f]^T xg[k,c])
            for f in range(KF):
                ps = ps1.tile([128, csz], mybir.dt.float32, tag="ps1")
                for k in range(KH):
                    nc.tensor.matmul(
                        ps[:],
                        w1t[k][:, f * 128:(f + 1) * 128],
                        xt[:, k, :],
                        start=(k == 0),
                        stop=(k == KH - 1),
                    )
                nc.scalar.activation(ht[:, f, :], ps[:], AF.Silu)
            # GEMM2: y[c, :] = (hT^T @ W2) * w_tok
            for m in range(-(-csz // 128)):
                mt = c0 // 128 + m
                mw = min(128, csz - m * 128)   # partial last token-tile
                msl = slice(m * 128, m * 128 + mw)
                pa = ps2.tile([128, 384], mybir.dt.float32, tag="psA")
                pb = ps2.tile([128, 384], mybir.dt.float32, tag="psB")
                for k in range(KF):
                    nc.tensor.matmul(pa[:mw, :], ht[:, k, msl],
                                     w2t[k][:, 0:384],
                                     start=(k == 0), stop=(k == KF - 1))
                for k in range(KF):
                    nc.tensor.matmul(pb[:mw, :], ht[:, k, msl],
                                     w2t[k][:, 384:768],
                                     start=(k == 0), stop=(k == KF - 1))
                yt = yp.tile([128, H], mybir.dt.float32, tag="y")
                nc.scalar.activation(yt[:mw, 0:384], pa[:mw, :], AF.Copy,
                                     scale=wta[:mw, mt:mt + 1])
                nc.scalar.activation(yt[:mw, 384:768], pb[:mw, :], AF.Copy,
                                     scale=wta[:mw, mt:mt + 1])
                nc.sync.dma_start(y[c0 + m * 128:c0 + m * 128 + mw, :],
                                  yt[:mw, :])

    nc.compile()
    return nc


def kernel(x: np.ndarray, Wg: np.ndarray, W1: np.ndarray, W2: np.ndarray):
    import ml_dtypes
    from concourse.bass_utils import run_bass_kernel_spmd

    x = np.asarray(x, dtype=np.float32)
    Wg = np.asarray(Wg, dtype=np.float32)
    W1 = np.asarray(W1, dtype=np.float32)
    W2 = np.asarray(W2, dtype=np.float32)
    x_flat = np.ascontiguousarray(x.reshape(T, H))

    top_idx, top_w = _route(x_flat, Wg)

    idx_lists = []
    wt_lists = []
    for e in range(E):
        sel = top_idx == e                       # [T, K] bool
        tok = np.nonzero(sel.any(axis=1))[0]     # tokens routed to e
        w_tok = (top_w * sel).sum(axis=1)[tok].astype(np.float32)
        idx_lists.append(tok)
        wt_lists.append(w_tok)

    max_cnt = max(len(t) for t in idx_lists)
    C = max(256, max_cnt)

    if C not in _COMPILED:
        _COMPILED[C] = _build(C)
    nc = _COMPILED[C]

    in_maps = []
    for e in range(E):
        tok = idx_lists[e]
        cnt = len(tok)
        g1dt = ml_dtypes.bfloat16 if G1_BF16 else np.float32
        xg = np.zeros((H, C), dtype=g1dt)
        xg[:, :cnt] = x_flat[tok].T.astype(g1dt)
        wt = np.zeros((-(-C // 128) * 128, 1), dtype=np.float32)
        wt[:cnt, 0] = wt_lists[e]
        in_maps.append({
            "xg": xg,
            "wt": wt,
            "w1": W1[e].astype(g1dt),
            "w2": W2[e].astype(ml_dtypes.bfloat16),
        })

    res = run_bass_kernel_spmd(
        nc, in_maps, core_ids=list(range(E)),
        trace=bool(globals().get("TRACE", False)),
    )
    globals()["LAST_RESULT"] = res

    y = np.zeros((T, H), dtype=np.float32)
    for e in range(E):
        tok = idx_lists[e]
        y[tok] += res.results[e]["y"][:len(tok)]

    return y.reshape(B, S, H), np.zeros((), dtype=np.float32)


# revision 19
# speedup vs baseline: 1.0368x; 1.0200x over previous
"""MoE layer (top-2 of 8 experts, H=768, F=3072, T=4096) on 8 TRN2 NeuronCores.

Strategy: expert parallelism with sparse dispatch.
  - Host computes the gate exactly as the reference does (jax on CPU:
    logits -> softmax -> top-2 -> renormalized top-2 weights) and builds the
    per-expert token lists ("all-to-all token dispatch" done host-side).
  - Core e receives: the tokens routed to expert e (gathered, transposed to
    [H, C] so both GEMMs need no on-device transposes), that expert's W1/W2,
    and the per-token combine weight.
  - Device computes y_e = (silu(x_e @ W1_e) @ W2_e) * w_tok. Both GEMMs run
    in bf16 (fp32 PSUM accumulation; G1_BF16=False switches GEMM1 to fp32r).
    Since the combine weight is a per-token scalar it commutes past W2 and is
    applied to the GEMM2 output on the scalar engine.
  - Host scatter-adds the two expert partials per token (combine/unshard).
"""

import os
import sys
from contextlib import ExitStack

for _p in ("/opt/trn_rl_repo",):
    if _p not in sys.path and os.path.isdir(_p):
        sys.path.insert(0, _p)

import numpy as np

B, S, H, F, E, TOPK = 2, 2048, 768, 3072, 8, 2
T = B * S
KH = H // 128   # 6  contraction chunks for GEMM1
KF = F // 128   # 24 contraction chunks for GEMM2
CHUNK = 384     # token chunk (moving-dim) for GEMM1
G1_BF16 = True  # GEMM1 in bf16 (vs fp32r)

_COMPILED = {}  # C -> (nc,)


def _route(x_flat: np.ndarray, Wg: np.ndarray):
    """Gate computed with the same ops/platform as the reference (jax CPU)."""
    try:
        import jax
        import jax.numpy as jnp

        cpu = jax.devices("cpu")[0]
        with jax.default_device(cpu):
            logits = jnp.asarray(x_flat) @ jnp.asarray(Wg)
            probs = jax.nn.softmax(logits, axis=-1)
            top_scores, top_idx = jax.lax.top_k(probs, TOPK)
            top_w = jax.nn.softmax(top_scores.astype(jnp.float32), axis=-1)
            return (np.asarray(top_idx), np.asarray(top_w, dtype=np.float32))
    except Exception:
        # numpy fallback (identical math; only fp summation order differs)
        logits = x_flat @ Wg
        z = logits - logits.max(axis=-1, keepdims=True)
        p = np.exp(z)
        probs = p / p.sum(axis=-1, keepdims=True)
        # top-k with ties broken toward lower index, like jax.lax.top_k
        order = np.argsort(-probs, axis=-1, kind="stable")
        top_idx = order[:, :TOPK].astype(np.int32)
        top_scores = np.take_along_axis(probs, top_idx, axis=-1)
        z2 = top_scores - top_scores.max(axis=-1, keepdims=True)
        p2 = np.exp(z2)
        top_w = (p2 / p2.sum(axis=-1, keepdims=True)).astype(np.float32)
        return top_idx, top_w


def _build(C: int):
    import concourse.tile as tile
    from concourse import bacc, mybir

    F32 = mybir.dt.float32
    F32R = mybir.dt.float32r
    BF16 = mybir.dt.bfloat16
    AF = mybir.ActivationFunctionType

    G1 = BF16 if G1_BF16 else F32R
    G1S = BF16 if G1_BF16 else F32  # storage dtype in DRAM

    nc = bacc.Bacc("TRN2", target_bir_lowering=False, debug=False)
    xg = nc.dram_tensor("xg", [H, C], G1S, kind="ExternalInput").ap()
    wt = nc.dram_tensor("wt", [-(-C // 128) * 128, 1], F32, kind="ExternalInput").ap()
    w1 = nc.dram_tensor("w1", [H, F], G1S, kind="ExternalInput").ap()
    w2 = nc.dram_tensor("w2", [F, H], BF16, kind="ExternalInput").ap()
    y = nc.dram_tensor("y", [C, H], F32, kind="ExternalOutput").ap()

    def g1cast(ap):
        return ap if G1_BF16 else ap.bitcast(F32R)

    NTOK = -(-C // 128)  # 128-tile count (last tile may be partial)
    chunks = []
    c0 = 0
    while c0 < C:
        csz = min(CHUNK, C - c0)
        chunks.append((c0, csz))
        c0 += csz

    with tile.TileContext(nc) as tc, ExitStack() as ctx:
        w1p = ctx.enter_context(tc.tile_pool(name="w1p", bufs=1))
        w2p = ctx.enter_context(tc.tile_pool(name="w2p", bufs=1))
        xp = ctx.enter_context(tc.tile_pool(name="xp", bufs=3))
        hp = ctx.enter_context(tc.tile_pool(name="hp", bufs=3))
        yp = ctx.enter_context(tc.tile_pool(name="yp", bufs=3))
        wtp = ctx.enter_context(tc.tile_pool(name="wtp", bufs=1))
        ps1 = ctx.enter_context(tc.tile_pool(name="ps1", bufs=4, space="PSUM"))
        ps2 = ctx.enter_context(tc.tile_pool(name="ps2", bufs=2, space="PSUM"))

        # x chunk 0 first, then W1 in f-eighths (chunk-0 GEMM1 starts after
        # only x0 + the first eighth of W1 has landed), x1 after the first
        # two eighths.
        xts = [xp.tile([128, KH, csz], G1, tag="x", name=f"x_{ci}")
               for ci, (c0, csz) in enumerate(chunks)]

        def load_x(ci):
            c0, csz = chunks[ci]
            nc.sync.dma_start(
                xts[ci][:],
                g1cast(xg[:, c0:c0 + csz]
                       .rearrange("(ko p) n -> p ko n", p=128)),
            )

        load_x(0)
        w1t = [w1p.tile([128, F], G1, tag=f"w1_{k}", name=f"w1_{k}")
               for k in range(KH)]
        for q in range(8):
            fsl = slice(q * (F // 8), (q + 1) * (F // 8))
            for k in range(KH):
                nc.sync.dma_start(
                    w1t[k][:, fsl],
                    g1cast(w1[k * 128:(k + 1) * 128, fsl]),
                )
            if q == 1 and len(chunks) > 1:
                load_x(1)
        # all combine weights: wta[p, n] = wt[n*128 + p]
        wta = wtp.tile([128, NTOK], F32, tag="wta")
        nc.sync.dma_start(wta[:], wt.rearrange("(n p) one -> p (n one)", p=128))
        w2t = [w2p.tile([128, H], BF16, tag=f"w2_{k}", name=f"w2_{k}")
               for k in range(KF)]
        for hh in range(2):
            hsl = slice(hh * 384, (hh + 1) * 384)
            for k in range(KF):
                nc.sync.dma_start(w2t[k][:, hsl], w2[k * 128:(k + 1) * 128, hsl])

        for ci, (c0, csz) in enumerate(chunks):
            xt = xts[ci]
            if ci >= 2:
                load_x(ci)
            ht = hp.tile([128, KF, csz], BF16, tag="h")
            # GEMM1: hT[f, c] = silu(sum_k W1[k,f]^T xg[k,c])
            for f in range(KF):
                ps = ps1.tile([128, csz], mybir.dt.float32, tag="ps1")
                for k in range(KH):
                    nc.tensor.matmul(
                        ps[:],
                        w1t[k][:, f * 128:(f + 1) * 128],
                        xt[:, k, :],
                        start=(k == 0),
                        stop=(k == KH - 1),
                    )
                nc.scalar.activation(ht[:, f, :], ps[:], AF.Silu)
            # GEMM2: y[c, :] = (hT^T @ W2) * w_tok
            for m in range(-(-csz // 128)):
                mt = c0 // 128 + m
                mw = min(128, csz - m * 128)   # partial last token-tile
                msl = slice(m * 128, m * 128 + mw)
                pa = ps2.tile([128, 384], mybir.dt.float32, tag="psA")
                pb = ps2.tile([128, 384], mybir.dt.float32, tag="psB")
                for k in range(KF):
                    nc.tensor.matmul(pa[:mw, :], ht[:, k, msl],
                                     w2t[k][:, 0:384],
                                     start=(k == 0), stop=(k == KF - 1))
                for k in range(KF):
                    nc.tensor.matmul(pb[:mw, :], ht[:, k, msl],
                                     w2t[k][:, 384:768],
                                     start=(k == 0), stop=(k == KF - 1))
                yt = yp.tile([128, H], mybir.dt.float32, tag="y")
                nc.scalar.activation(yt[:mw, 0:384], pa[:mw, :], AF.Copy,
                                     scale=wta[:mw, mt:mt + 1])
                nc.scalar.activation(yt[:mw, 384:768], pb[:mw, :], AF.Copy,
                                     scale=wta[:mw, mt:mt + 1])
                nc.sync.dma_start(y[c0 + m * 128:c0 + m * 128 + mw, :],
                                  yt[:mw, :])

    nc.compile()
    return nc


def _build_v4(C: int):
    """All-resident-hT structure: GEMM2 streams tokens as the moving dim
    (cost scales with C exactly), output is yT [H, C], combine weight applied
    on the vector engine from a host-broadcast [128, C] tile."""
    import concourse.tile as tile
    from concourse import bacc, mybir

    F32 = mybir.dt.float32
    F32R = mybir.dt.float32r
    BF16 = mybir.dt.bfloat16
    AF = mybir.ActivationFunctionType

    G1 = BF16 if G1_BF16 else F32R
    G1S = BF16 if G1_BF16 else F32

    nc = bacc.Bacc("TRN2", target_bir_lowering=False, debug=False)
    xg = nc.dram_tensor("xg", [H, C], G1S, kind="ExternalInput").ap()
    wbc = nc.dram_tensor("wbc", [128, C], F32, kind="ExternalInput").ap()
    w1 = nc.dram_tensor("w1", [H, F], G1S, kind="ExternalInput").ap()
    w2 = nc.dram_tensor("w2", [F, H], BF16, kind="ExternalInput").ap()
    yT = nc.dram_tensor("yT", [H, C], F32, kind="ExternalOutput").ap()

    def g1cast(ap):
        return ap if G1_BF16 else ap.bitcast(F32R)

    chunks = []
    c0 = 0
    while c0 < C:
        csz = min(CHUNK, C - c0)
        chunks.append((c0, csz))
        c0 += csz

    with tile.TileContext(nc) as tc, ExitStack() as ctx:
        w1p = ctx.enter_context(tc.tile_pool(name="w1p", bufs=1))
        w2p = ctx.enter_context(tc.tile_pool(name="w2p", bufs=1))
        xp = ctx.enter_context(tc.tile_pool(name="xp", bufs=3))
        hp = ctx.enter_context(tc.tile_pool(name="hp", bufs=1))
        yp = ctx.enter_context(tc.tile_pool(name="yp", bufs=4))
        wtp = ctx.enter_context(tc.tile_pool(name="wtp", bufs=1))

        # x chunk 0 first, then W1 in f-eighths, x1 after two eighths.
        xts = [xp.tile([128, KH, csz], G1, tag="x", name=f"x_{ci}")
               for ci, (c0, csz) in enumerate(chunks)]

        def load_x(ci):
            c0, csz = chunks[ci]
            nc.sync.dma_start(
                xts[ci][:],
                g1cast(xg[:, c0:c0 + csz]
                       .rearrange("(ko p) n -> p ko n", p=128)),
            )

        load_x(0)
        w1t = [w1p.tile([128, F], G1, tag=f"w1_{k}", name=f"w1_{k}")
               for k in range(KH)]
        for q in range(8):
            fsl = slice(q * (F // 8), (q + 1) * (F // 8))
            for k in range(KH):
                nc.sync.dma_start(
                    w1t[k][:, fsl],
                    g1cast(w1[k * 128:(k + 1) * 128, fsl]),
                )
            if q == 1 and len(chunks) > 1:
                load_x(1)
        wbt = wtp.tile([128, C], F32, tag="wbt")
        nc.sync.dma_start(wbt[:], wbc[:])
        w2t = [w2p.tile([128, H], BF16, tag=f"w2_{k}", name=f"w2_{k}")
               for k in range(KF)]
        for hh in range(2):
            hsl = slice(hh * 384, (hh + 1) * 384)
            for k in range(KF):
                nc.sync.dma_start(w2t[k][:, hsl], w2[k * 128:(k + 1) * 128, hsl])

        hts = [hp.tile([128, KF, csz], BF16, tag=f"h_{ci}", name=f"h_{ci}")
               for ci, (c0, csz) in enumerate(chunks)]

        # GEMM1: hT[f, c] = silu(sum_k W1[k,f]^T xg[k,c]) per chunk
        with tc.tile_pool(name="ps1", bufs=4, space="PSUM") as ps1:
            for ci, (c0, csz) in enumerate(chunks):
                xt = xts[ci]
                if ci >= 2:
                    load_x(ci)
                ht = hts[ci]
                for f in range(KF):
                    ps = ps1.tile([128, csz], mybir.dt.float32, tag="ps1")
                    for k in range(KH):
                        nc.tensor.matmul(
                            ps[:],
                            w1t[k][:, f * 128:(f + 1) * 128],
                            xt[:, k, :],
                            start=(k == 0),
                            stop=(k == KH - 1),
                        )
                    nc.scalar.activation(ht[:, f, :], ps[:], AF.Silu)

        # GEMM2: yT[hb, c] = (sum_k W2[k, hb]^T hT[k, c]) * w[c]
        with tc.tile_pool(name="psY", bufs=6, space="PSUM") as psY:
            for hb in range(H // 128):
                hsl = slice(hb * 128, (hb + 1) * 128)
                pss = [psY.tile([128, csz], mybir.dt.float32, tag="psY",
                                name=f"psY_{hb}_{ci}")
                       for ci, (c0, csz) in enumerate(chunks)]
                for k in range(KF):
                    for ci, (c0, csz) in enumerate(chunks):
                        nc.tensor.matmul(pss[ci][:], w2t[k][:, hsl],
                                         hts[ci][:, k, :],
                                         start=(k == 0), stop=(k == KF - 1))
                for ci, (c0, csz) in enumerate(chunks):
                    yt = yp.tile([128, csz], mybir.dt.float32, tag="y")
                    nc.vector.tensor_mul(yt[:], pss[ci][:],
                                         wbt[:, c0:c0 + csz])
                    nc.sync.dma_start(yT[hsl, c0:c0 + csz], yt[:])

    nc.compile()
    return nc


def kernel(x: np.ndarray, Wg: np.ndarray, W1: np.ndarray, W2: np.ndarray):
    import ml_dtypes
    from concourse.bass_utils import run_bass_kernel_spmd

    x = np.asarray(x, dtype=np.float32)
    Wg = np.asarray(Wg, dtype=np.float32)
    W1 = np.asarray(W1, dtype=np.float32)
    W2 = np.asarray(W2, dtype=np.float32)
    x_flat = np.ascontiguousarray(x.reshape(T, H))

    top_idx, top_w = _route(x_flat, Wg)

    idx_lists = []
    wt_lists = []
    for e in range(E):
        sel = top_idx == e                       # [T, K] bool
        tok = np.nonzero(sel.any(axis=1))[0]     # tokens routed to e
        w_tok = (top_w * sel).sum(axis=1)[tok].astype(np.float32)
        idx_lists.append(tok)
        wt_lists.append(w_tok)

    max_cnt = max(len(t) for t in idx_lists)
    C = max(256, max_cnt)
    v4 = C <= 1792  # hT for all chunks must fit SBUF

    key = (C, v4)
    if key not in _COMPILED:
        _COMPILED[key] = _build_v4(C) if v4 else _build(C)
    nc = _COMPILED[key]

    in_maps = []
    for e in range(E):
        tok = idx_lists[e]
        cnt = len(tok)
        g1dt = ml_dtypes.bfloat16 if G1_BF16 else np.float32
        xg = np.zeros((H, C), dtype=g1dt)
        xg[:, :cnt] = x_flat[tok].T.astype(g1dt)
        m = {
            "xg": xg,
            "w1": W1[e].astype(g1dt),
            "w2": W2[e].astype(ml_dtypes.bfloat16),
        }
        if v4:
            wrow = np.zeros((C,), dtype=np.float32)
            wrow[:cnt] = wt_lists[e]
            m["wbc"] = np.ascontiguousarray(
                np.broadcast_to(wrow[None, :], (128, C)))
        else:
            wt = np.zeros((-(-C // 128) * 128, 1), dtype=np.float32)
            wt[:cnt, 0] = wt_lists[e]
            m["wt"] = wt
        in_maps.append(m)

    res = run_bass_kernel_spmd(
        nc, in_maps, core_ids=list(range(E)),
        trace=bool(globals().get("TRACE", False)),
    )
    globals()["LAST_RESULT"] = res

    y = np.zeros((T, H), dtype=np.float32)
    for e in range(E):
        tok = idx_lists[e]
        if v4:
            y[tok] += res.results[e]["yT"][:, :len(tok)].T
        else:
            y[tok] += res.results[e]["y"][:len(tok)]

    return y.reshape(B, S, H), np.zeros((), dtype=np.float32)


# revision 20
# speedup vs baseline: 1.0679x; 1.0301x over previous
"""MoE layer (top-2 of 8 experts, H=768, F=3072, T=4096) on 8 TRN2 NeuronCores.

Strategy: expert parallelism with sparse dispatch.
  - Host computes the gate exactly as the reference does (jax on CPU:
    logits -> softmax -> top-2 -> renormalized top-2 weights) and builds the
    per-expert token lists ("all-to-all token dispatch" done host-side).
  - Core e receives: the tokens routed to expert e (gathered, transposed to
    [H, C] so both GEMMs need no on-device transposes), that expert's W1/W2,
    and the per-token combine weight.
  - Device computes y_e = (silu(x_e @ W1_e) @ W2_e) * w_tok. Both GEMMs run
    in bf16 (fp32 PSUM accumulation; G1_BF16=False switches GEMM1 to fp32r).
    Since the combine weight is a per-token scalar it commutes past W2 and is
    applied to the GEMM2 output on the scalar engine.
  - Host scatter-adds the two expert partials per token (combine/unshard).
"""

import os
import sys
from contextlib import ExitStack

for _p in ("/opt/trn_rl_repo",):
    if _p not in sys.path and os.path.isdir(_p):
        sys.path.insert(0, _p)

import numpy as np

B, S, H, F, E, TOPK = 2, 2048, 768, 3072, 8, 2
T = B * S
KH = H // 128   # 6  contraction chunks for GEMM1
KF = F // 128   # 24 contraction chunks for GEMM2
CHUNK = 384     # token chunk (moving-dim) for GEMM1
G1_BF16 = True  # GEMM1 in bf16 (vs fp32r)

_COMPILED = {}  # C -> (nc,)


def _route(x_flat: np.ndarray, Wg: np.ndarray):
    """Gate computed with the same ops/platform as the reference (jax CPU)."""
    try:
        import jax
        import jax.numpy as jnp

        cpu = jax.devices("cpu")[0]
        with jax.default_device(cpu):
            logits = jnp.asarray(x_flat) @ jnp.asarray(Wg)
            probs = jax.nn.softmax(logits, axis=-1)
            top_scores, top_idx = jax.lax.top_k(probs, TOPK)
            top_w = jax.nn.softmax(top_scores.astype(jnp.float32), axis=-1)
            return (np.asarray(top_idx), np.asarray(top_w, dtype=np.float32))
    except Exception:
        # numpy fallback (identical math; only fp summation order differs)
        logits = x_flat @ Wg
        z = logits - logits.max(axis=-1, keepdims=True)
        p = np.exp(z)
        probs = p / p.sum(axis=-1, keepdims=True)
        # top-k with ties broken toward lower index, like jax.lax.top_k
        order = np.argsort(-probs, axis=-1, kind="stable")
        top_idx = order[:, :TOPK].astype(np.int32)
        top_scores = np.take_along_axis(probs, top_idx, axis=-1)
        z2 = top_scores - top_scores.max(axis=-1, keepdims=True)
        p2 = np.exp(z2)
        top_w = (p2 / p2.sum(axis=-1, keepdims=True)).astype(np.float32)
        return top_idx, top_w


def _build(C: int):
    import concourse.tile as tile
    from concourse import bacc, mybir

    F32 = mybir.dt.float32
    F32R = mybir.dt.float32r
    BF16 = mybir.dt.bfloat16
    AF = mybir.ActivationFunctionType

    G1 = BF16 if G1_BF16 else F32R
    G1S = BF16 if G1_BF16 else F32  # storage dtype in DRAM

    nc = bacc.Bacc("TRN2", target_bir_lowering=False, debug=False)
    xg = nc.dram_tensor("xg", [H, C], G1S, kind="ExternalInput").ap()
    wt = nc.dram_tensor("wt", [-(-C // 128) * 128, 1], F32, kind="ExternalInput").ap()
    w1 = nc.dram_tensor("w1", [H, F], G1S, kind="ExternalInput").ap()
    w2 = nc.dram_tensor("w2", [F, H], BF16, kind="ExternalInput").ap()
    y = nc.dram_tensor("y", [C, H], F32, kind="ExternalOutput").ap()

    def g1cast(ap):
        return ap if G1_BF16 else ap.bitcast(F32R)

    NTOK = -(-C // 128)  # 128-tile count (last tile may be partial)
    chunks = []
    c0 = 0
    while c0 < C:
        csz = min(CHUNK, C - c0)
        chunks.append((c0, csz))
        c0 += csz

    with tile.TileContext(nc) as tc, ExitStack() as ctx:
        w1p = ctx.enter_context(tc.tile_pool(name="w1p", bufs=1))
        w2p = ctx.enter_context(tc.tile_pool(name="w2p", bufs=1))
        xp = ctx.enter_context(tc.tile_pool(name="xp", bufs=3))
        hp = ctx.enter_context(tc.tile_pool(name="hp", bufs=3))
        yp = ctx.enter_context(tc.tile_pool(name="yp", bufs=3))
        wtp = ctx.enter_context(tc.tile_pool(name="wtp", bufs=1))
        ps1 = ctx.enter_context(tc.tile_pool(name="ps1", bufs=4, space="PSUM"))
        ps2 = ctx.enter_context(tc.tile_pool(name="ps2", bufs=2, space="PSUM"))

        # x chunk 0 first, then W1 in f-eighths (chunk-0 GEMM1 starts after
        # only x0 + the first eighth of W1 has landed), x1 after the first
        # two eighths.
        xts = [xp.tile([128, KH, csz], G1, tag="x", name=f"x_{ci}")
               for ci, (c0, csz) in enumerate(chunks)]

        def load_x(ci):
            c0, csz = chunks[ci]
            nc.sync.dma_start(
                xts[ci][:],
                g1cast(xg[:, c0:c0 + csz]
                       .rearrange("(ko p) n -> p ko n", p=128)),
            )

        load_x(0)
        w1t = [w1p.tile([128, F], G1, tag=f"w1_{k}", name=f"w1_{k}")
               for k in range(KH)]
        for q in range(8):
            fsl = slice(q * (F // 8), (q + 1) * (F // 8))
            for k in range(KH):
                nc.sync.dma_start(
                    w1t[k][:, fsl],
                    g1cast(w1[k * 128:(k + 1) * 128, fsl]),
                )
            if q == 1 and len(chunks) > 1:
                load_x(1)
        # all combine weights: wta[p, n] = wt[n*128 + p]
        wta = wtp.tile([128, NTOK], F32, tag="wta")
        nc.sync.dma_start(wta[:], wt.rearrange("(n p) one -> p (n one)", p=128))
        w2t = [w2p.tile([128, H], BF16, tag=f"w2_{k}", name=f"w2_{k}")
               for k in range(KF)]
        for hh in range(2):
            hsl = slice(hh * 384, (hh + 1) * 384)
            for k in range(KF):
                nc.sync.dma_start(w2t[k][:, hsl], w2[k * 128:(k + 1) * 128, hsl])

        for ci, (c0, csz) in enumerate(chunks):
            xt = xts[ci]
            if ci >= 2:
                load_x(ci)
            ht = hp.tile([128, KF, csz], BF16, tag="h")
            # GEMM1: hT[f, c] = silu(sum_k W1[k,f]^T xg[k,c])
            for f in range(KF):
                ps = ps1.tile([128, csz], mybir.dt.float32, tag="ps1")
                for k in range(KH):
                    nc.tensor.matmul(
                        ps[:],
                        w1t[k][:, f * 128:(f + 1) * 128],
                        xt[:, k, :],
                        start=(k == 0),
                        stop=(k == KH - 1),
                    )
                nc.scalar.activation(ht[:, f, :], ps[:], AF.Silu)
            # GEMM2: y[c, :] = (hT^T @ W2) * w_tok
            for m in range(-(-csz // 128)):
                mt = c0 // 128 + m
                mw = min(128, csz - m * 128)   # partial last token-tile
                msl = slice(m * 128, m * 128 + mw)
                pa = ps2.tile([128, 384], mybir.dt.float32, tag="psA")
                pb = ps2.tile([128, 384], mybir.dt.float32, tag="psB")
                for k in range(KF):
                    nc.tensor.matmul(pa[:mw, :], ht[:, k, msl],
                                     w2t[k][:, 0:384],
                                     start=(k == 0), stop=(k == KF - 1))
                for k in range(KF):
                    nc.tensor.matmul(pb[:mw, :], ht[:, k, msl],
                                     w2t[k][:, 384:768],
                                     start=(k == 0), stop=(k == KF - 1))
                yt = yp.tile([128, H], mybir.dt.float32, tag="y")
                nc.scalar.activation(yt[:mw, 0:384], pa[:mw, :], AF.Copy,
                                     scale=wta[:mw, mt:mt + 1])
                nc.scalar.activation(yt[:mw, 384:768], pb[:mw, :], AF.Copy,
                                     scale=wta[:mw, mt:mt + 1])
                nc.sync.dma_start(y[c0 + m * 128:c0 + m * 128 + mw, :],
                                  yt[:mw, :])

    nc.compile()
    return nc


def _build_v4(C: int):
    """All-resident-hT structure: GEMM2 streams tokens as the moving dim
    (cost scales with C exactly), output is yT [H, C], combine weight applied
    on the vector engine from a host-broadcast [128, C] tile."""
    import concourse.tile as tile
    from concourse import bacc, mybir

    F32 = mybir.dt.float32
    F32R = mybir.dt.float32r
    BF16 = mybir.dt.bfloat16
    AF = mybir.ActivationFunctionType

    G1 = BF16 if G1_BF16 else F32R
    G1S = BF16 if G1_BF16 else F32

    nc = bacc.Bacc("TRN2", target_bir_lowering=False, debug=False)
    xg = nc.dram_tensor("xg", [H, C], G1S, kind="ExternalInput").ap()
    wbc = nc.dram_tensor("wbc", [128, C], F32, kind="ExternalInput").ap()
    w1 = nc.dram_tensor("w1", [H, F], G1S, kind="ExternalInput").ap()
    w2 = nc.dram_tensor("w2", [F, H], BF16, kind="ExternalInput").ap()
    yT = nc.dram_tensor("yT", [H, C], F32, kind="ExternalOutput").ap()

    def g1cast(ap):
        return ap if G1_BF16 else ap.bitcast(F32R)

    chunks = []
    c0 = 0
    while c0 < C:
        csz = min(CHUNK, C - c0)
        chunks.append((c0, csz))
        c0 += csz

    with tile.TileContext(nc) as tc, ExitStack() as ctx:
        w1p = ctx.enter_context(tc.tile_pool(name="w1p", bufs=1))
        w2p = ctx.enter_context(tc.tile_pool(name="w2p", bufs=1))
        xp = ctx.enter_context(tc.tile_pool(name="xp", bufs=3))
        hp = ctx.enter_context(tc.tile_pool(name="hp", bufs=1))
        yp = ctx.enter_context(tc.tile_pool(name="yp", bufs=4))
        wtp = ctx.enter_context(tc.tile_pool(name="wtp", bufs=1))

        # x chunk 0 first, then W1 in f-eighths, x1 after two eighths.
        xts = [xp.tile([128, KH, csz], G1, tag="x", name=f"x_{ci}")
               for ci, (c0, csz) in enumerate(chunks)]

        def load_x(ci):
            c0, csz = chunks[ci]
            nc.sync.dma_start(
                xts[ci][:],
                g1cast(xg[:, c0:c0 + csz]
                       .rearrange("(ko p) n -> p ko n", p=128)),
            )

        load_x(0)
        # W1 as one tile, loaded in 4 f-quarter strided DMAs on the sync
        # HWDGE ring; W2 + wbc go on the scalar HWDGE ring in parallel.
        w1all = w1p.tile([128, KH, F], G1, tag="w1all")
        w1r = w1.rearrange("(ko p) f -> p ko f", p=128)
        for q in range(4):
            fsl = slice(q * (F // 4), (q + 1) * (F // 4))
            nc.sync.dma_start(w1all[:, :, fsl], g1cast(w1r[:, :, fsl]))
            if q == 0 and len(chunks) > 1:
                load_x(1)
        w2all = w2p.tile([128, KF, H], BF16, tag="w2all")
        nc.scalar.dma_start(w2all[:], w2.rearrange("(ko p) n -> p ko n", p=128))
        wbt = wtp.tile([128, C], F32, tag="wbt")
        nc.scalar.dma_start(wbt[:], wbc[:])

        hts = [hp.tile([128, KF, csz], BF16, tag=f"h_{ci}", name=f"h_{ci}")
               for ci, (c0, csz) in enumerate(chunks)]

        # GEMM1: hT[f, c] = silu(sum_k W1[k,f]^T xg[k,c]) per chunk
        with tc.tile_pool(name="ps1", bufs=4, space="PSUM") as ps1:
            for ci, (c0, csz) in enumerate(chunks):
                xt = xts[ci]
                if ci >= 2:
                    load_x(ci)
                ht = hts[ci]
                for f in range(KF):
                    ps = ps1.tile([128, csz], mybir.dt.float32, tag="ps1")
                    for k in range(KH):
                        nc.tensor.matmul(
                            ps[:],
                            w1all[:, k, f * 128:(f + 1) * 128],
                            xt[:, k, :],
                            start=(k == 0),
                            stop=(k == KH - 1),
                        )
                    nc.scalar.activation(ht[:, f, :], ps[:], AF.Silu)

        # GEMM2: yT[hb, c] = (sum_k W2[k, hb]^T hT[k, c]) * w[c]
        with tc.tile_pool(name="psY", bufs=6, space="PSUM") as psY:
            for hb in range(H // 128):
                hsl = slice(hb * 128, (hb + 1) * 128)
                pss = [psY.tile([128, csz], mybir.dt.float32, tag="psY",
                                name=f"psY_{hb}_{ci}")
                       for ci, (c0, csz) in enumerate(chunks)]
                for k in range(KF):
                    for ci, (c0, csz) in enumerate(chunks):
                        nc.tensor.matmul(pss[ci][:], w2all[:, k, hsl],
                                         hts[ci][:, k, :],
                                         start=(k == 0), stop=(k == KF - 1))
                for ci, (c0, csz) in enumerate(chunks):
                    yt = yp.tile([128, csz], mybir.dt.float32, tag="y")
                    nc.vector.tensor_mul(yt[:], pss[ci][:],
                                         wbt[:, c0:c0 + csz])
                    nc.gpsimd.dma_start(yT[hsl, c0:c0 + csz], yt[:])

    nc.compile()
    return nc


def kernel(x: np.ndarray, Wg: np.ndarray, W1: np.ndarray, W2: np.ndarray):
    import ml_dtypes
    from concourse.bass_utils import run_bass_kernel_spmd

    x = np.asarray(x, dtype=np.float32)
    Wg = np.asarray(Wg, dtype=np.float32)
    W1 = np.asarray(W1, dtype=np.float32)
    W2 = np.asarray(W2, dtype=np.float32)
    x_flat = np.ascontiguousarray(x.reshape(T, H))

    top_idx, top_w = _route(x_flat, Wg)

    idx_lists = []
    wt_lists = []
    for e in range(E):
        sel = top_idx == e                       # [T, K] bool
        tok = np.nonzero(sel.any(axis=1))[0]     # tokens routed to e
        w_tok = (top_w * sel).sum(axis=1)[tok].astype(np.float32)
        idx_lists.append(tok)
        wt_lists.append(w_tok)

    max_cnt = max(len(t) for t in idx_lists)
    C = max(256, max_cnt)
    v4 = C <= 1792  # hT for all chunks must fit SBUF

    key = (C, v4)
    if key not in _COMPILED:
        _COMPILED[key] = _build_v4(C) if v4 else _build(C)
    nc = _COMPILED[key]

    in_maps = []
    for e in range(E):
        tok = idx_lists[e]
        cnt = len(tok)
        g1dt = ml_dtypes.bfloat16 if G1_BF16 else np.float32
        xg = np.zeros((H, C), dtype=g1dt)
        xg[:, :cnt] = x_flat[tok].T.astype(g1dt)
        m = {
            "xg": xg,
            "w1": W1[e].astype(g1dt),
            "w2": W2[e].astype(ml_dtypes.bfloat16),
        }
        if v4:
            wrow = np.zeros((C,), dtype=np.float32)
            wrow[:cnt] = wt_lists[e]
            m["wbc"] = np.ascontiguousarray(
                np.broadcast_to(wrow[None, :], (128, C)))
        else:
            wt = np.zeros((-(-C // 128) * 128, 1), dtype=np.float32)
            wt[:cnt, 0] = wt_lists[e]
            m["wt"] = wt
        in_maps.append(m)

    res = run_bass_kernel_spmd(
        nc, in_maps, core_ids=list(range(E)),
        trace=bool(globals().get("TRACE", False)),
    )
    globals()["LAST_RESULT"] = res

    y = np.zeros((T, H), dtype=np.float32)
    for e in range(E):
        tok = idx_lists[e]
        if v4:
            y[tok] += res.results[e]["yT"][:, :len(tok)].T
        else:
            y[tok] += res.results[e]["y"][:len(tok)]

    return y.reshape(B, S, H), np.zeros((), dtype=np.float32)


# revision 21
# speedup vs baseline: 1.1237x; 1.0522x over previous
"""MoE layer (top-2 of 8 experts, H=768, F=3072, T=4096) on 8 TRN2 NeuronCores.

Strategy: expert parallelism with sparse dispatch.
  - Host computes the gate exactly as the reference does (jax on CPU:
    logits -> softmax -> top-2 -> renormalized top-2 weights) and builds the
    per-expert token lists ("all-to-all token dispatch" done host-side).
  - Core e receives: the tokens routed to expert e (gathered, transposed to
    [H, C] so both GEMMs need no on-device transposes), that expert's W1/W2,
    and the per-token combine weight.
  - Device computes y_e = (silu(x_e @ W1_e) @ W2_e) * w_tok. Both GEMMs run
    in bf16 (fp32 PSUM accumulation; G1_BF16=False switches GEMM1 to fp32r).
    Since the combine weight is a per-token scalar it commutes past W2 and is
    applied to the GEMM2 output on the scalar engine.
  - Host scatter-adds the two expert partials per token (combine/unshard).
"""

import os
import sys
from contextlib import ExitStack

for _p in ("/opt/trn_rl_repo",):
    if _p not in sys.path and os.path.isdir(_p):
        sys.path.insert(0, _p)

import numpy as np

B, S, H, F, E, TOPK = 2, 2048, 768, 3072, 8, 2
T = B * S
KH = H // 128   # 6  contraction chunks for GEMM1
KF = F // 128   # 24 contraction chunks for GEMM2
CHUNK = 384     # token chunk (moving-dim) for GEMM1
G1_BF16 = True  # GEMM1 in bf16 (vs fp32r)

_COMPILED = {}  # C -> (nc,)


def _route(x_flat: np.ndarray, Wg: np.ndarray):
    """Gate computed with the same ops/platform as the reference (jax CPU)."""
    try:
        import jax
        import jax.numpy as jnp

        cpu = jax.devices("cpu")[0]
        with jax.default_device(cpu):
            logits = jnp.asarray(x_flat) @ jnp.asarray(Wg)
            probs = jax.nn.softmax(logits, axis=-1)
            top_scores, top_idx = jax.lax.top_k(probs, TOPK)
            top_w = jax.nn.softmax(top_scores.astype(jnp.float32), axis=-1)
            return (np.asarray(top_idx), np.asarray(top_w, dtype=np.float32))
    except Exception:
        # numpy fallback (identical math; only fp summation order differs)
        logits = x_flat @ Wg
        z = logits - logits.max(axis=-1, keepdims=True)
        p = np.exp(z)
        probs = p / p.sum(axis=-1, keepdims=True)
        # top-k with ties broken toward lower index, like jax.lax.top_k
        order = np.argsort(-probs, axis=-1, kind="stable")
        top_idx = order[:, :TOPK].astype(np.int32)
        top_scores = np.take_along_axis(probs, top_idx, axis=-1)
        z2 = top_scores - top_scores.max(axis=-1, keepdims=True)
        p2 = np.exp(z2)
        top_w = (p2 / p2.sum(axis=-1, keepdims=True)).astype(np.float32)
        return top_idx, top_w


def _build(C: int):
    import concourse.tile as tile
    from concourse import bacc, mybir

    F32 = mybir.dt.float32
    F32R = mybir.dt.float32r
    BF16 = mybir.dt.bfloat16
    AF = mybir.ActivationFunctionType

    G1 = BF16 if G1_BF16 else F32R
    G1S = BF16 if G1_BF16 else F32  # storage dtype in DRAM

    nc = bacc.Bacc("TRN2", target_bir_lowering=False, debug=False)
    xg = nc.dram_tensor("xg", [H, C], G1S, kind="ExternalInput").ap()
    wt = nc.dram_tensor("wt", [-(-C // 128) * 128, 1], F32, kind="ExternalInput").ap()
    w1 = nc.dram_tensor("w1", [H, F], G1S, kind="ExternalInput").ap()
    w2 = nc.dram_tensor("w2", [F, H], BF16, kind="ExternalInput").ap()
    y = nc.dram_tensor("y", [C, H], F32, kind="ExternalOutput").ap()

    def g1cast(ap):
        return ap if G1_BF16 else ap.bitcast(F32R)

    NTOK = -(-C // 128)  # 128-tile count (last tile may be partial)
    chunks = []
    c0 = 0
    while c0 < C:
        csz = min(CHUNK, C - c0)
        chunks.append((c0, csz))
        c0 += csz

    with tile.TileContext(nc) as tc, ExitStack() as ctx:
        w1p = ctx.enter_context(tc.tile_pool(name="w1p", bufs=1))
        w2p = ctx.enter_context(tc.tile_pool(name="w2p", bufs=1))
        xp = ctx.enter_context(tc.tile_pool(name="xp", bufs=3))
        hp = ctx.enter_context(tc.tile_pool(name="hp", bufs=3))
        yp = ctx.enter_context(tc.tile_pool(name="yp", bufs=3))
        wtp = ctx.enter_context(tc.tile_pool(name="wtp", bufs=1))
        ps1 = ctx.enter_context(tc.tile_pool(name="ps1", bufs=4, space="PSUM"))
        ps2 = ctx.enter_context(tc.tile_pool(name="ps2", bufs=2, space="PSUM"))

        # x chunk 0 first, then W1 in f-eighths (chunk-0 GEMM1 starts after
        # only x0 + the first eighth of W1 has landed), x1 after the first
        # two eighths.
        xts = [xp.tile([128, KH, csz], G1, tag="x", name=f"x_{ci}")
               for ci, (c0, csz) in enumerate(chunks)]

        def load_x(ci):
            c0, csz = chunks[ci]
            nc.sync.dma_start(
                xts[ci][:],
                g1cast(xg[:, c0:c0 + csz]
                       .rearrange("(ko p) n -> p ko n", p=128)),
            )

        load_x(0)
        w1t = [w1p.tile([128, F], G1, tag=f"w1_{k}", name=f"w1_{k}")
               for k in range(KH)]
        for q in range(8):
            fsl = slice(q * (F // 8), (q + 1) * (F // 8))
            for k in range(KH):
                nc.sync.dma_start(
                    w1t[k][:, fsl],
                    g1cast(w1[k * 128:(k + 1) * 128, fsl]),
                )
            if q == 1 and len(chunks) > 1:
                load_x(1)
        # all combine weights: wta[p, n] = wt[n*128 + p]
        wta = wtp.tile([128, NTOK], F32, tag="wta")
        nc.sync.dma_start(wta[:], wt.rearrange("(n p) one -> p (n one)", p=128))
        w2t = [w2p.tile([128, H], BF16, tag=f"w2_{k}", name=f"w2_{k}")
               for k in range(KF)]
        for hh in range(2):
            hsl = slice(hh * 384, (hh + 1) * 384)
            for k in range(KF):
                nc.sync.dma_start(w2t[k][:, hsl], w2[k * 128:(k + 1) * 128, hsl])

        for ci, (c0, csz) in enumerate(chunks):
            xt = xts[ci]
            if ci >= 2:
                load_x(ci)
            ht = hp.tile([128, KF, csz], BF16, tag="h")
            # GEMM1: hT[f, c] = silu(sum_k W1[k,f]^T xg[k,c])
            for f in range(KF):
                ps = ps1.tile([128, csz], mybir.dt.float32, tag="ps1")
                for k in range(KH):
                    nc.tensor.matmul(
                        ps[:],
                        w1t[k][:, f * 128:(f + 1) * 128],
                        xt[:, k, :],
                        start=(k == 0),
                        stop=(k == KH - 1),
                    )
                nc.scalar.activation(ht[:, f, :], ps[:], AF.Silu)
            # GEMM2: y[c, :] = (hT^T @ W2) * w_tok
            for m in range(-(-csz // 128)):
                mt = c0 // 128 + m
                mw = min(128, csz - m * 128)   # partial last token-tile
                msl = slice(m * 128, m * 128 + mw)
                pa = ps2.tile([128, 384], mybir.dt.float32, tag="psA")
                pb = ps2.tile([128, 384], mybir.dt.float32, tag="psB")
                for k in range(KF):
                    nc.tensor.matmul(pa[:mw, :], ht[:, k, msl],
                                     w2t[k][:, 0:384],
                                     start=(k == 0), stop=(k == KF - 1))
                for k in range(KF):
                    nc.tensor.matmul(pb[:mw, :], ht[:, k, msl],
                                     w2t[k][:, 384:768],
                                     start=(k == 0), stop=(k == KF - 1))
                yt = yp.tile([128, H], mybir.dt.float32, tag="y")
                nc.scalar.activation(yt[:mw, 0:384], pa[:mw, :], AF.Copy,
                                     scale=wta[:mw, mt:mt + 1])
                nc.scalar.activation(yt[:mw, 384:768], pb[:mw, :], AF.Copy,
                                     scale=wta[:mw, mt:mt + 1])
                nc.sync.dma_start(y[c0 + m * 128:c0 + m * 128 + mw, :],
                                  yt[:mw, :])

    nc.compile()
    return nc


def _build_v4(C: int):
    """All-resident-hT structure: GEMM2 streams tokens as the moving dim
    (cost scales with C exactly), output is yT [H, C], combine weight applied
    on the vector engine from a host-broadcast [128, C] tile."""
    import concourse.tile as tile
    from concourse import bacc, mybir

    F32 = mybir.dt.float32
    F32R = mybir.dt.float32r
    BF16 = mybir.dt.bfloat16
    AF = mybir.ActivationFunctionType

    G1 = BF16 if G1_BF16 else F32R
    G1S = BF16 if G1_BF16 else F32

    nc = bacc.Bacc("TRN2", target_bir_lowering=False, debug=False)
    xg = nc.dram_tensor("xg", [H, C], G1S, kind="ExternalInput").ap()
    wbc = nc.dram_tensor("wbc", [128, C], F32, kind="ExternalInput").ap()
    w1 = nc.dram_tensor("w1", [H, F], G1S, kind="ExternalInput").ap()
    w2 = nc.dram_tensor("w2", [F, H], BF16, kind="ExternalInput").ap()
    yT = nc.dram_tensor("yT", [H, C], F32, kind="ExternalOutput").ap()

    def g1cast(ap):
        return ap if G1_BF16 else ap.bitcast(F32R)

    chunks = []
    c0 = 0
    while c0 < C:
        csz = min(CHUNK, C - c0)
        chunks.append((c0, csz))
        c0 += csz

    with tile.TileContext(nc) as tc, ExitStack() as ctx:
        w1p = ctx.enter_context(tc.tile_pool(name="w1p", bufs=1))
        w2p = ctx.enter_context(tc.tile_pool(name="w2p", bufs=1))
        xp = ctx.enter_context(tc.tile_pool(name="xp", bufs=3))
        hp = ctx.enter_context(tc.tile_pool(name="hp", bufs=1))
        yp = ctx.enter_context(tc.tile_pool(name="yp", bufs=4))
        wtp = ctx.enter_context(tc.tile_pool(name="wtp", bufs=1))

        # x chunk 0 first, then W1 in f-eighths, x1 after two eighths.
        xts = [xp.tile([128, KH, csz], G1, tag="x", name=f"x_{ci}")
               for ci, (c0, csz) in enumerate(chunks)]

        def load_x(ci):
            c0, csz = chunks[ci]
            nc.gpsimd.dma_start(
                xts[ci][:],
                g1cast(xg[:, c0:c0 + csz]
                       .rearrange("(ko p) n -> p ko n", p=128)),
            )

        # x chunks ride the gpsimd SWDGE ring; W1 f-quarters alternate
        # between the two HWDGE rings (sync + scalar) so they land in
        # parallel; W2 + wbc follow on the scalar ring.
        load_x(0)
        if len(chunks) > 1:
            load_x(1)
        w1all = w1p.tile([128, KH, F], G1, tag="w1all")
        w1r = w1.rearrange("(ko p) f -> p ko f", p=128)
        for q in range(4):
            fsl = slice(q * (F // 4), (q + 1) * (F // 4))
            eng = nc.sync if q % 2 == 0 else nc.scalar
            eng.dma_start(w1all[:, :, fsl], g1cast(w1r[:, :, fsl]))
        w2all = w2p.tile([128, KF, H], BF16, tag="w2all")
        nc.scalar.dma_start(w2all[:], w2.rearrange("(ko p) n -> p ko n", p=128))
        wbt = wtp.tile([128, C], F32, tag="wbt")
        nc.sync.dma_start(wbt[:], wbc[:])

        hts = [hp.tile([128, KF, csz], BF16, tag=f"h_{ci}", name=f"h_{ci}")
               for ci, (c0, csz) in enumerate(chunks)]

        # GEMM1: hT[f, c] = silu(sum_k W1[k,f]^T xg[k,c]) per chunk
        with tc.tile_pool(name="ps1", bufs=4, space="PSUM") as ps1:
            for ci, (c0, csz) in enumerate(chunks):
                xt = xts[ci]
                if ci >= 2:
                    load_x(ci)
                ht = hts[ci]
                for f in range(KF):
                    ps = ps1.tile([128, csz], mybir.dt.float32, tag="ps1")
                    for k in range(KH):
                        nc.tensor.matmul(
                            ps[:],
                            w1all[:, k, f * 128:(f + 1) * 128],
                            xt[:, k, :],
                            start=(k == 0),
                            stop=(k == KH - 1),
                        )
                    nc.scalar.activation(ht[:, f, :], ps[:], AF.Silu)

        # GEMM2: yT[hb, c] = (sum_k W2[k, hb]^T hT[k, c]) * w[c]
        with tc.tile_pool(name="psY", bufs=6, space="PSUM") as psY:
            for hb in range(H // 128):
                hsl = slice(hb * 128, (hb + 1) * 128)
                pss = [psY.tile([128, csz], mybir.dt.float32, tag="psY",
                                name=f"psY_{hb}_{ci}")
                       for ci, (c0, csz) in enumerate(chunks)]
                for k in range(KF):
                    for ci, (c0, csz) in enumerate(chunks):
                        nc.tensor.matmul(pss[ci][:], w2all[:, k, hsl],
                                         hts[ci][:, k, :],
                                         start=(k == 0), stop=(k == KF - 1))
                for ci, (c0, csz) in enumerate(chunks):
                    yt = yp.tile([128, csz], mybir.dt.float32, tag="y")
                    nc.vector.tensor_mul(yt[:], pss[ci][:],
                                         wbt[:, c0:c0 + csz])
                    nc.gpsimd.dma_start(yT[hsl, c0:c0 + csz], yt[:])

    nc.compile()
    return nc


def kernel(x: np.ndarray, Wg: np.ndarray, W1: np.ndarray, W2: np.ndarray):
    import ml_dtypes
    from concourse.bass_utils import run_bass_kernel_spmd

    x = np.asarray(x, dtype=np.float32)
    Wg = np.asarray(Wg, dtype=np.float32)
    W1 = np.asarray(W1, dtype=np.float32)
    W2 = np.asarray(W2, dtype=np.float32)
    x_flat = np.ascontiguousarray(x.reshape(T, H))

    top_idx, top_w = _route(x_flat, Wg)

    idx_lists = []
    wt_lists = []
    for e in range(E):
        sel = top_idx == e                       # [T, K] bool
        tok = np.nonzero(sel.any(axis=1))[0]     # tokens routed to e
        w_tok = (top_w * sel).sum(axis=1)[tok].astype(np.float32)
        idx_lists.append(tok)
        wt_lists.append(w_tok)

    max_cnt = max(len(t) for t in idx_lists)
    C = max(256, max_cnt)
    v4 = C <= 1792  # hT for all chunks must fit SBUF

    key = (C, v4)
    if key not in _COMPILED:
        _COMPILED[key] = _build_v4(C) if v4 else _build(C)
    nc = _COMPILED[key]

    in_maps = []
    for e in range(E):
        tok = idx_lists[e]
        cnt = len(tok)
        g1dt = ml_dtypes.bfloat16 if G1_BF16 else np.float32
        xg = np.zeros((H, C), dtype=g1dt)
        xg[:, :cnt] = x_flat[tok].T.astype(g1dt)
        m = {
            "xg": xg,
            "w1": W1[e].astype(g1dt),
            "w2": W2[e].astype(ml_dtypes.bfloat16),
        }
        if v4:
            wrow = np.zeros((C,), dtype=np.float32)
            wrow[:cnt] = wt_lists[e]
            m["wbc"] = np.ascontiguousarray(
                np.broadcast_to(wrow[None, :], (128, C)))
        else:
            wt = np.zeros((-(-C // 128) * 128, 1), dtype=np.float32)
            wt[:cnt, 0] = wt_lists[e]
            m["wt"] = wt
        in_maps.append(m)

    res = run_bass_kernel_spmd(
        nc, in_maps, core_ids=list(range(E)),
        trace=bool(globals().get("TRACE", False)),
    )
    globals()["LAST_RESULT"] = res

    y = np.zeros((T, H), dtype=np.float32)
    for e in range(E):
        tok = idx_lists[e]
        if v4:
            y[tok] += res.results[e]["yT"][:, :len(tok)].T
        else:
            y[tok] += res.results[e]["y"][:len(tok)]

    return y.reshape(B, S, H), np.zeros((), dtype=np.float32)
